# revision 1
# baseline (speedup 1.0000x reference)
"""Trainium2 Bass kernel for CrossAttention (B=4, QL=KL=2048, D=1024, fp32).

reference:
    query = hidden_states @ Wq                      # [B, QL, D]
    kv    = decoder_hidden_states @ Wkv             # [B, KL, 2D]
    key, value = split(kv, 2, axis=-1)
    scores = einsum('bqd,bkd->bqk', query, key) / sqrt(D)
    w = softmax(scores, axis=-1)
    out = einsum('bqk,bkd->bqd', w, value)          # [B, QL, D]

Sharding: 8 cores = batch(4) x q-half(2).  Each core owns 1024 query rows of
one batch and computes the full K/V projection for its batch (KV work
duplicated x2 across the pair sharing a batch; no collectives needed).

All matmuls run in float32r (TF32-like), which streams at full PE rate for
moving dims >= 256.  Softmax runs without max-subtraction (scores here are
~N(0,1); exp stays far from fp32 limits) using ACT's fused exp(scale*x) with
accum_out row sums.  P^T for the AV matmul is built with DVE 32x32 stream
transposes (sbuf->sbuf), and the attention loop is software-pipelined so PE
runs scores(q+1) while DVE transposes P(q).

Phase order QT -> KT -> V -> attention.  SBUF is managed on two allocation
stacks (long-lived pools right, transient pools left) so later phases'
weights prefetch during earlier phases' compute.  DMA issue order is
critical-first: each phase's first-needed chunk is issued before background
prefetch, and bulk tensors move as single multi-block DMAs (one SWDGE
trigger, 4KB descriptor rows).

This walrus build allows only ONE embedded semaphore wait per hardware
instruction; legalize_waits() splits any extra waits onto injected
same-engine NOPs after Tile scheduling.
"""

import sys

if "/opt/trn_rl_repo" not in sys.path:
    sys.path.insert(0, "/opt/trn_rl_repo")

import numpy as np

import bass_rust
import concourse.bass as bass
import concourse.mybir as mybir
import concourse.tile as tile
from concourse.bass_utils import run_bass_kernel_spmd

F32 = mybir.dt.float32
F32R = mybir.dt.float32r
EXP = mybir.ActivationFunctionType.Exp
ACOPY = mybir.ActivationFunctionType.Copy

N_CORES = 8
B, QL, KL, D = 4, 2048, 2048, 1024


def legalize_waits(nc, max_waits=1):
    """TRN2 instructions embed at most one semaphore wait.  Move excess waits
    emitted by Tile onto same-engine NOPs inserted just before the owning
    instruction (engine FIFO makes this semantically identical)."""
    cnt = 0
    for fn in nc.m.functions:
        for bb in fn.blocks:
            out = []
            changed = False
            for ins in bb.instructions:
                si = ins.sync_info
                if si is not None and si.on_wait and len(si.on_wait) > max_waits:
                    waits = list(si.on_wait)
                    for w in waits[:-max_waits]:
                        cnt += 1
                        nop = bass_rust.InstNoOp(name=f"I-wfix-{cnt}")
                        nop.engine = ins.engine
                        nop.sync_info = mybir.SyncInfo(on_wait=[w], on_update=[])
                        out.append(nop)
                    ins.sync_info = mybir.SyncInfo(
                        on_wait=waits[-max_waits:],
                        on_update=list(si.on_update or []),
                    )
                    changed = True
                out.append(ins)
            if changed:
                bb.instructions = out
    return cnt


def build_attention(nc, QS, KLp, Dp, scale):
    DS = Dp // 128          # contraction subtiles
    NDO = Dp // 128         # output-d 128-chunks
    NKC = KLp // 512        # k 512-chunks (scores)
    NKT = KLp // 128        # k 128-chunks
    NQT = QS // 128         # q tiles
    NDC = Dp // 512         # d 512-chunks (AV / Wkv_hi)
    NA1 = KLp // 512        # A1 rhs 512-chunks
    NQC = QS // 512         # B rhs 512-chunks
    BLK = DS * 128          # free extent of one [128, DS*128] DRAM block

    # block-layout params: [nblk, 128, DS*128]
    hsT = nc.declare_dram_parameter("hsT", [NQT, 128, BLK], F32R, isOutput=False)
    decT = nc.declare_dram_parameter("decT", [NKT, 128, BLK], F32R, isOutput=False)
    wq = nc.declare_dram_parameter("wq", [NDO, 128, BLK], F32R, isOutput=False)
    wkv = nc.declare_dram_parameter("wkv", [2 * NDO, 128, BLK], F32R, isOutput=False)
    out = nc.declare_dram_parameter("out", [QS, Dp], F32, isOutput=True)

    def load_blocks(dst, src, blk0, nblk):
        """One DMA moving nblk consecutive [128, BLK] DRAM blocks into an
        SBUF tile laid out [128, DS, nblk, 128] (or [128, DS, 128] if 1)."""
        if nblk == 1:
            nc.sync.dma_start(
                dst[:], src[blk0].rearrange("p (s o) -> p s o", o=128)
            )
        else:
            nc.sync.dma_start(
                dst.rearrange("p b s o -> p b (s o)"),
                src[blk0 : blk0 + nblk].rearrange("b p f -> p b f"),
            )

    with tile.TileContext(nc) as tc:
        # Two SBUF allocation stacks: long-lived pools (identity, KT, V,
        # q-tiles, attention working set) on the RIGHT stack close at the
        # end; transient per-phase + prefetch pools on the LEFT stack close
        # LIFO at phase boundaries.
        pools = []

        def enter(cm):
            pools.append(cm)
            return cm.__enter__()

        def close(cm):
            pools.remove(cm)
            cm.__exit__(None, None, None)

        constp_cm = tc.tile_pool(name="const", bufs=1, side="right")
        dramp_cm = tc.tile_pool(name="dram", bufs=1, space="DRAM")
        whip_cm = tc.tile_pool(name="whi", bufs=1)
        dt2p_cm = tc.tile_pool(name="dt2", bufs=3)
        wlop_cm = tc.tile_pool(name="wlo", bufs=1)
        dt1p_cm = tc.tile_pool(name="dt1", bufs=2)
        wqp_cm = tc.tile_pool(name="wqp", bufs=1)
        htp_cm = tc.tile_pool(name="hst", bufs=2)
        stgp_cm = tc.tile_pool(name="stg", bufs=4)
        psB_cm = tc.tile_pool(name="psB", bufs=3, space="PSUM")

        constp = enter(constp_cm)
        dramp = enter(dramp_cm)
        whip = enter(whip_cm)
        dt2p = enter(dt2p_cm)
        wlop = enter(wlop_cm)
        dt1p = enter(dt1p_cm)
        wqp = enter(wqp_cm)
        htp = enter(htp_cm)
        stgp = enter(stgp_cm)
        psB = enter(psB_cm)

        ident = constp.tile([128, 128], F32)
        nc.gpsimd.memset(ident[:], 0.0)
        nc.gpsimd.affine_select(
            out=ident[:], in_=ident[:],
            compare_op=mybir.AluOpType.not_equal,
            fill=1.0, base=0, pattern=[[-1, 128]], channel_multiplier=1,
        )
        qt_dram = dramp.tile([NQC, 128, DS, 512], F32R)

        # HAM warmup: keep the PE busy during the initial DMA wave so the
        # clock gate is at 8/8 when phase B's first real matmul issues.
        warm = constp.tile([128, 640], F32R)
        nc.vector.tensor_copy(warm[:], ident[:, 0:1].to_broadcast([128, 640]))
        warm_ps_cm = tc.tile_pool(name="wps", bufs=1, space="PSUM")
        warm_ps_pool = enter(warm_ps_cm)
        warm_ps = warm_ps_pool.tile([128, 512], F32)
        for _ in range(70):
            nc.tensor.matmul(
                warm_ps[:], warm[:, 0:128], warm[:, 128:640],
                start=True, stop=True, skip_group_check=True,
            )

        # reserve the prefetch tiles up-front (left stack, stable addresses);
        # their DMAs are issued later, behind B's critical loads
        whi = whip.tile([128, NDO, DS, 128], F32R, tag="whi")
        wlo = wlop.tile([128, NDO, DS, 128], F32R, tag="wlo")
        dt1s = {}
        for kc in range(min(2, NA1)):
            dt1s[kc] = dt1p.tile([128, 4, DS, 128], F32R, tag="dt1", name=f"dt1_{kc}")

        close(warm_ps_cm)

        # ---- critical-first loads: B's first groups, then the rest of wq ---
        wqt = wqp.tile([128, NDO, DS, 128], F32R, tag="wqp")
        load_blocks(wqt[:, 0:2], wq, 0, 2)
        hts = []
        ht0 = htp.tile([128, 4, DS, 128], F32R, tag="hst", name="ht0")
        load_blocks(ht0[:], hsT, 0, 4)
        hts.append(ht0)
        if NDO > 5:
            load_blocks(wqt[:, 2:5], wq, 2, 3)
        ht1 = None
        if NQC > 1:
            ht1 = htp.tile([128, 4, DS, 128], F32R, tag="hst", name="ht1")
            load_blocks(ht1[:], hsT, 4, 4)
            hts.append(ht1)
        if NDO > 5:
            load_blocks(wqt[:, 5:NDO], wq, 5, NDO - 5)
        else:
            load_blocks(wqt[:, 2:NDO], wq, 2, NDO - 2)

        # ---------------- Phase B: QT[do, q] = Wq^T @ hsT -> DRAM -----------
        for qc in range(NQC):
            if 0 < qc < NQC - 1:
                ht = htp.tile([128, 4, DS, 128], F32R, tag="hst", name=f"ht{qc+1}")
                load_blocks(ht[:], hsT, 4 * (qc + 1), 4)
                hts.append(ht)
            for do in range(NDO):
                if qc == NQC - 1:
                    # background prefetch for A1, spread across B's last wave
                    if do == 1:
                        load_blocks(wlo[:], wkv, 0, NDO)
                    elif do == 3 and 0 in dt1s:
                        load_blocks(dt1s[0][:], decT, 0, 4)
                    elif do == 5 and 1 in dt1s:
                        load_blocks(dt1s[1][:], decT, 4, 4)
                ps = psB.tile([128, 512], F32, tag="psB")
                for di in range(DS):
                    nc.tensor.matmul(
                        ps[:], wqt[:, do, di, :], hts[qc][:, :, di, :],
                        start=(di == 0), stop=(di == DS - 1),
                    )
                st = stgp.tile([128, 512], F32R, tag="stg")
                nc.vector.tensor_copy(st[:], ps[:])
                nc.sync.dma_start(qt_dram[qc, :, do, :], st[:])
        if NDO <= 5 and 1 in dt1s:
            # small-config catch-up: B's last wave had no do==5 slot
            load_blocks(dt1s[1][:], decT, 4, 4)
        close(psB_cm)
        close(stgp_cm)
        close(htp_cm)
        close(wqp_cm)

        # ---------------- Phase A1: KT[do, k] = Wkv_lo^T @ decT -------------
        ktp_cm = tc.tile_pool(name="ktp", bufs=1, side="right")
        qtp_cm = tc.tile_pool(name="qt", bufs=3, side="right")
        psA_cm = tc.tile_pool(name="psA", bufs=3, space="PSUM")
        ktp = enter(ktp_cm)
        qtp = enter(qtp_cm)
        psA = enter(psA_cm)
        KT = ktp.tile([128, DS, KLp], F32R, tag="KT")   # [d, k] rhs for scores
        qtiles = {}
        dt2s = {}

        for kc in range(NA1):
            if kc + 2 < NA1:
                t = dt1p.tile([128, 4, DS, 128], F32R, tag="dt1", name=f"dt1_{kc+2}")
                load_blocks(t[:], decT, 4 * (kc + 2), 4)
                dt1s[kc + 2] = t
            if kc == 1:
                # prefetch A2's weights under A1's compute
                load_blocks(whi[:], wkv, NDO, NDO)
            if kc == NA1 - 1:
                for kt in range(min(3, NKT)):
                    t = dt2p.tile([128, DS, 128], F32R, tag="dt2", name=f"dt2_{kt}")
                    load_blocks(t, decT, kt, 1)
                    dt2s[kt] = t
            dt = dt1s[kc]
            for do in range(NDO):
                ps = psA.tile([128, 512], F32, tag="psA")
                for di in range(DS):
                    nc.tensor.matmul(
                        ps[:], wlo[:, do, di, :], dt[:, :, di, :],
                        start=(di == 0), stop=(di == DS - 1),
                    )
                nc.vector.tensor_copy(
                    KT[:, do, kc * 512 : (kc + 1) * 512], ps[:]
                )
        close(psA_cm)
        close(dt1p_cm)
        close(wlop_cm)

        # ---------------- Phase A2: V[k, d] = decT^T @ Wkv_hi ---------------
        vp_cm = tc.tile_pool(name="vp", bufs=1, side="right")
        psV_cm = tc.tile_pool(name="psV", bufs=3, space="PSUM")
        vp = enter(vp_cm)
        psV = enter(psV_cm)
        V = vp.tile([128, NKT, Dp], F32R, tag="V")       # [k, d] rhs for AV
        for kt in range(NKT):
            if kt + 3 < NKT:
                t = dt2p.tile([128, DS, 128], F32R, tag="dt2", name=f"dt2_{kt+3}")
                load_blocks(t, decT, kt + 3, 1)
                dt2s[kt + 3] = t
            if kt == NKT - 2:
                # prefetch first attention q-tiles (qt_dram fully written)
                for qt in range(min(2, NQT)):
                    qtile = qtp.tile(
                        [128, DS, 128], F32R, tag="qt", name=f"qtile{qt}"
                    )
                    nc.sync.dma_start(
                        qtile[:],
                        qt_dram[qt // 4][:, :, (qt % 4) * 128 : (qt % 4 + 1) * 128],
                    )
                    qtiles[qt] = qtile
            dt = dt2s[kt]
            for dc in range(NDC):
                ps = psV.tile([128, 512], F32, tag="psV")
                for di in range(DS):
                    nc.tensor.matmul(
                        ps[:], dt[:, di, :], whi[:, 4 * dc : 4 * (dc + 1), di, :],
                        start=(di == 0), stop=(di == DS - 1),
                    )
                nc.vector.tensor_copy(
                    V[:, kt, dc * 512 : (dc + 1) * 512], ps[:]
                )
        close(psV_cm)
        close(dt2p_cm)
        close(whip_cm)

        # ---------------- Phase C: attention per q-tile ---------------------
        pp_cm = tc.tile_pool(name="pp", bufs=2, side="right")
        ptp1_cm = tc.tile_pool(name="ptp1", bufs=1, side="right")
        ptp_cm = tc.tile_pool(name="ptp", bufs=2, side="right")
        statp_cm = tc.tile_pool(name="stat", bufs=NQT, side="right")
        ostp_cm = tc.tile_pool(name="ost", bufs=2, side="right")
        ps_sc_cm = tc.tile_pool(name="ps_sc", bufs=5, space="PSUM")
        ps_av_cm = tc.tile_pool(name="ps_av", bufs=3, space="PSUM")
        pp = enter(pp_cm)
        ptp1 = enter(ptp1_cm)
        ptp = enter(ptp_cm)
        statp = enter(statp_cm)
        ostp = enter(ostp_cm)
        ps_sc = enter(ps_sc_cm)
        ps_av = enter(ps_av_cm)

        def emit_scores(qt):
            """scores + exp + row-sum stats for q-tile qt."""
            qtile = qtiles[qt]
            P = pp.tile([128, NKT, 128], F32, tag="pp", name=f"P{qt}")
            lpart = statp.tile([128, NKC + 1], F32, tag="stat", name=f"lp{qt}")
            for kc in range(NKC):
                ps = ps_sc.tile([128, 512], F32, tag="ps_sc")
                for di in range(DS):
                    nc.tensor.matmul(
                        ps[:], qtile[:, di, :],
                        KT[:, di, kc * 512 : (kc + 1) * 512],
                        start=(di == 0), stop=(di == DS - 1),
                    )
                nc.scalar.activation(
                    P[:, 4 * kc : 4 * (kc + 1), :], ps[:], EXP,
                    bias=0.0, scale=float(scale),
                    accum_out=lpart[:, kc : kc + 1],
                )
            return P, lpart

        def emit_softmax_stats(lpart, qt):
            nc.vector.tensor_tensor(
                lpart[:, NKC : NKC + 1], lpart[:, 0:1], lpart[:, 1:2],
                mybir.AluOpType.add,
            )
            for kc in range(2, NKC):
                nc.vector.tensor_tensor(
                    lpart[:, NKC : NKC + 1], lpart[:, NKC : NKC + 1],
                    lpart[:, kc : kc + 1], mybir.AluOpType.add,
                )
            recip = statp.tile([128, 1], F32, tag="recip", name=f"rc{qt}")
            nc.vector.reciprocal(recip[:], lpart[:, NKC : NKC + 1])
            return recip

        def emit_transposes(P, qt):
            """PT[k, kt, q] = P[q, kt, k].T per kt: DVE 32x32 stream blocks
            (f32), then one rounding copy to f32r for the AV matmul."""
            PT1 = ptp1.tile([128, NKT, 128], F32, tag="ptp1", name=f"PT1_{qt}")
            for a in range(4):
                for c in range(4):
                    nc.vector.transpose(
                        PT1[32 * c : 32 * c + 32, :, 32 * a : 32 * a + 32],
                        P[32 * a : 32 * a + 32, :, 32 * c : 32 * c + 32],
                    )
            PT = ptp.tile([128, NKT, 128], F32R, tag="ptp", name=f"PT{qt}")
            nc.gpsimd.tensor_copy(PT[:], PT1[:])
            return PT

        def emit_av(qt, PT, recip):
            avs = [
                ps_av.tile([128, 512], F32, tag="ps_av", name=f"av{qt}_{i}")
                for i in range(NDC)
            ]
            for kt in range(NKT):
                for dc in range(NDC):
                    nc.tensor.matmul(
                        avs[dc][:], PT[:, kt, :],
                        V[:, kt, dc * 512 : (dc + 1) * 512],
                        start=(kt == 0), stop=(kt == NKT - 1),
                    )
            ot = ostp.tile([128, Dp], F32, tag="ost")
            for dc in range(NDC):
                nc.scalar.activation(
                    ot[:, dc * 512 : (dc + 1) * 512], avs[dc][:],
                    ACOPY, bias=0.0, scale=recip[:],
                )
            nc.sync.dma_start(out[qt * 128 : (qt + 1) * 128, :], ot[:])

        def emit_av_petr(qt, P, recip):
            """last-tile path: PE transposes feed AV directly (no DVE dep)."""
            PT = ptp.tile([128, NKT, 128], F32R, tag="ptp", name=f"PTz{qt}")
            avs = [
                ps_av.tile([128, 512], F32, tag="ps_av", name=f"avz{qt}_{i}")
                for i in range(NDC)
            ]
            for kt in range(NKT):
                pst = ps_sc.tile([128, 128], F32, tag="ps_sc", name=f"pst{kt}")
                nc.tensor.transpose(pst[:], P[:, kt, :], ident[:])
                nc.vector.tensor_copy(PT[:, kt, :], pst[:])
                for dc in range(NDC):
                    nc.tensor.matmul(
                        avs[dc][:], PT[:, kt, :],
                        V[:, kt, dc * 512 : (dc + 1) * 512],
                        start=(kt == 0), stop=(kt == NKT - 1),
                    )
            ot = ostp.tile([128, Dp], F32, tag="ost")
            for dc in range(NDC):
                nc.scalar.activation(
                    ot[:, dc * 512 : (dc + 1) * 512], avs[dc][:],
                    ACOPY, bias=0.0, scale=recip[:],
                )
            nc.sync.dma_start(out[qt * 128 : (qt + 1) * 128, :], ot[:])

        # software pipeline: PE runs scores(q+1) while DVE transposes P(q)
        state = {}
        for qt in range(NQT):
            if qt + 2 < NQT:
                qtile = qtp.tile([128, DS, 128], F32R, tag="qt", name=f"qtile{qt+2}")
                nc.sync.dma_start(
                    qtile[:],
                    qt_dram[(qt + 2) // 4][
                        :, :, ((qt + 2) % 4) * 128 : ((qt + 2) % 4 + 1) * 128
                    ],
                )
                qtiles[qt + 2] = qtile
            P, lpart = emit_scores(qt)
            recip = emit_softmax_stats(lpart, qt)
            if qt == NQT - 1:
                if qt > 0:
                    emit_av(qt - 1, *state.pop(qt - 1))
                emit_av_petr(qt, P, recip)
            else:
                PT = emit_transposes(P, qt)
                state[qt] = (PT, recip)
                if qt > 0:
                    emit_av(qt - 1, *state.pop(qt - 1))

        for cm in list(reversed(pools)):
            close(cm)

    legalize_waits(nc)
    return nc


def _pack_dT_blocks(x, DS):
    """[N, Dp] -> [N//128, 128, DS*128] where block b holds
    res[b, p, s*128+o] = x[b*128+o, s*128+p]  (partitions carry d, free
    carries (subtile s, n-within-block))."""
    N, Dp = x.shape
    r = x.reshape(N // 128, 128, DS, 128).transpose(0, 3, 2, 1)
    return np.ascontiguousarray(r.reshape(N // 128, 128, DS * 128))


def prepare_in_maps(hidden_states, decoder_hidden_states, Wq, Wkv):
    hidden_states = np.asarray(hidden_states, dtype=np.float32)
    decoder_hidden_states = np.asarray(decoder_hidden_states, dtype=np.float32)
    Wq = np.asarray(Wq, dtype=np.float32)
    Wkv = np.asarray(Wkv, dtype=np.float32)
    QS = QL // 2
    DS = D // 128

    wq_p = _pack_dT_blocks(Wq.T, DS)      # [do][p, s*128+o] = Wq[s*128+p, do*128+o]
    wkv_p = _pack_dT_blocks(Wkv.T, DS)

    in_maps = []
    for c in range(N_CORES):
        b, h = c // 2, c % 2
        hs = hidden_states[b, h * QS : (h + 1) * QS]        # [QS, D]
        dec = decoder_hidden_states[b]                      # [KL, D]
        in_maps.append(
            {
                "hsT": _pack_dT_blocks(hs, DS),    # [NQT, 128, DS*128]
                "decT": _pack_dT_blocks(dec, DS),  # [NKT, 128, DS*128]
                "wq": wq_p,
                "wkv": wkv_p,
            }
        )
    return in_maps


def kernel(hidden_states, decoder_hidden_states, Wq, Wkv):
    QS = QL // 2
    scale = 1.0 / float(np.sqrt(D))

    nc = bass.Bass()
    build_attention(nc, QS, KL, D, scale)
    in_maps = prepare_in_maps(hidden_states, decoder_hidden_states, Wq, Wkv)

    res = run_bass_kernel_spmd(nc, in_maps, list(range(N_CORES)))

    out = np.empty((B, QL, D), dtype=np.float32)
    for c in range(N_CORES):
        b, h = c // 2, c % 2
        out[b, h * QS : (h + 1) * QS] = res.results[c]["out"]
    return out



# revision 8
# speedup vs baseline: 1.4401x; 1.4401x over previous
"""Trainium2 Bass kernel for CrossAttention (B=4, QL=KL=2048, D=1024, fp32).

reference:
    query = hidden_states @ Wq                      # [B, QL, D]
    kv    = decoder_hidden_states @ Wkv             # [B, KL, 2D]
    key, value = split(kv, 2, axis=-1)
    scores = einsum('bqd,bkd->bqk', query, key) / sqrt(D)
    w = softmax(scores, axis=-1)
    out = einsum('bqk,bkd->bqd', w, value)          # [B, QL, D]

Sharding: 8 cores = batch(4) x q-half(2).  Each core owns 1024 query rows of
one batch and computes the full K/V projection for its batch (KV work
duplicated x2 across the pair sharing a batch; no collectives needed).

All data is fp16 (11-bit mantissa: ~0.05% quantization error, well inside the
2e-2 gate).  fp16 matmuls stream at 1 row/cycle like fp32r but their weight
loads use the fast-weight-load path (2 elems / 32-bit read) and half the
SBUF/DMA footprint, so everything stays resident in SBUF (no QT DRAM
roundtrip).

Attention is computed with TRANSPOSED scores: S^T[k, q] = K @ Q^T directly
via lhsT=KT-slice [d,k], rhs=QT [d, q-512].  exp(S^T) on ACT lands in exactly
the [k, q] stationary layout the AV matmul needs, so there are NO DVE
transposes and NO f32r casts anywhere.  Row sums for softmax come from one
extra 1-column matmul against a ones vector that reuses the AV stationary,
and normalization is folded into the ACT copy that drains AV psum.

Phase order: warmup -> B (Q proj) -> A1 (K proj) -> A2 (V proj) -> C
(scores both q-chunks, then AV).  DMA is critical-first: B's first weights,
then bulk prefetch of later phases' tensors behind B/A1 compute.

This walrus build allows only ONE embedded semaphore wait per hardware
instruction; legalize_waits() splits any extra waits onto injected
same-engine NOPs after Tile scheduling.
"""

import sys

if "/opt/trn_rl_repo" not in sys.path:
    sys.path.insert(0, "/opt/trn_rl_repo")

import numpy as np

import bass_rust
import concourse.bass as bass
import concourse.mybir as mybir
import concourse.tile as tile
from concourse.bass_utils import run_bass_kernel_spmd

F32 = mybir.dt.float32
F16 = mybir.dt.float16
EXP = mybir.ActivationFunctionType.Exp
ACOPY = mybir.ActivationFunctionType.Copy

N_CORES = 8
B, QL, KL, D = 4, 2048, 2048, 1024


def legalize_waits(nc, max_waits=1):
    """TRN2 instructions embed at most one semaphore wait.  Move excess waits
    emitted by Tile onto same-engine NOPs inserted just before the owning
    instruction (engine FIFO makes this semantically identical)."""
    cnt = 0
    for fn in nc.m.functions:
        for bb in fn.blocks:
            out = []
            changed = False
            for ins in bb.instructions:
                si = ins.sync_info
                if si is not None and si.on_wait and len(si.on_wait) > max_waits:
                    waits = list(si.on_wait)
                    for w in waits[:-max_waits]:
                        cnt += 1
                        nop = bass_rust.InstNoOp(name=f"I-wfix-{cnt}")
                        nop.engine = ins.engine
                        nop.sync_info = mybir.SyncInfo(on_wait=[w], on_update=[])
                        out.append(nop)
                    ins.sync_info = mybir.SyncInfo(
                        on_wait=waits[-max_waits:],
                        on_update=list(si.on_update or []),
                    )
                    changed = True
                out.append(ins)
            if changed:
                bb.instructions = out
    return cnt


def build_attention(nc, QS, KLp, Dp, scale):
    DS = Dp // 128          # contraction subtiles
    NDO = Dp // 128         # output-d 128-chunks
    NKT = KLp // 128        # k 128-chunks
    NQT = QS // 128         # q 128-chunks
    NQC = QS // 512         # q 512-chunks
    BLK = DS * 128          # free extent of one [128, DS*128] DRAM block

    # block-layout params: [nblk, 128, DS*128], fp16
    hsT = nc.declare_dram_parameter("hsT", [NQT, 128, BLK], F16, isOutput=False)
    decT = nc.declare_dram_parameter("decT", [NKT, 128, BLK], F16, isOutput=False)
    wq = nc.declare_dram_parameter("wq", [NDO, 128, BLK], F16, isOutput=False)
    wkv = nc.declare_dram_parameter("wkv", [2 * NDO, 128, BLK], F16, isOutput=False)
    out = nc.declare_dram_parameter("out", [QS, Dp], F32, isOutput=True)

    def load_blocks(dst, src, blk0, nblk):
        """One DMA moving nblk consecutive [128, BLK] DRAM blocks into an
        SBUF tile laid out [128, nblk, DS, 128]."""
        if nblk == 1:
            nc.sync.dma_start(
                dst[:], src[blk0].rearrange("p (s o) -> p s o", o=128)
            )
        else:
            nc.sync.dma_start(
                dst.rearrange("p b s o -> p b (s o)"),
                src[blk0 : blk0 + nblk].rearrange("b p f -> p b f"),
            )

    with tile.TileContext(nc) as tc:
        pools = []

        def enter(cm):
            pools.append(cm)
            return cm.__enter__()

        def close(cm):
            pools.remove(cm)
            cm.__exit__(None, None, None)

        # long-lived pools on the RIGHT stack; transient per-phase pools on
        # the LEFT stack close LIFO at phase boundaries.
        constp_cm = tc.tile_pool(name="const", bufs=1, side="right")
        ktp_cm = tc.tile_pool(name="ktp", bufs=1, side="right")
        vp_cm = tc.tile_pool(name="vp", bufs=1, side="right")
        qtp_cm = tc.tile_pool(name="qtp", bufs=1, side="right")

        wqp_cm = tc.tile_pool(name="wqp", bufs=1)
        htp_cm = tc.tile_pool(name="htp", bufs=1)
        wlop_cm = tc.tile_pool(name="wlo", bufs=1)
        whip_cm = tc.tile_pool(name="whi", bufs=1)
        decp_cm = tc.tile_pool(name="dec", bufs=1)

        psP_cm = tc.tile_pool(name="psP", bufs=4, space="PSUM")

        constp = enter(constp_cm)
        ktp = enter(ktp_cm)
        vp = enter(vp_cm)
        qtp = enter(qtp_cm)
        wqp = enter(wqp_cm)
        htp = enter(htp_cm)
        wlop = enter(wlop_cm)
        whip = enter(whip_cm)
        decp = enter(decp_cm)
        psP = enter(psP_cm)

        # constants: ones column (for row sums) + warmup tile
        ones = constp.tile([128, 8], F16)
        nc.gpsimd.memset(ones[:], 1.0)
        warm = constp.tile([128, 640], F16)
        nc.gpsimd.memset(warm[:], 1.0)

        # HAM warmup: keep the PE busy during the initial DMA wave so the
        # clock gate is at 8/8 when phase B's first real matmul issues.
        warm_ps_cm = tc.tile_pool(name="wps", bufs=1, space="PSUM")
        warm_ps_pool = enter(warm_ps_cm)
        warm_ps = warm_ps_pool.tile([128, 512], F32)
        for _ in range(70):
            nc.tensor.matmul(
                warm_ps[:], warm[:, 0:128], warm[:, 128:640],
                start=True, stop=True, skip_group_check=True,
            )
        close(warm_ps_cm)

        # ---- critical-first loads: B's weights + hsT, then bulk prefetch --
        wqt = wqp.tile([128, NDO, DS, 128], F16, tag="wqp")
        ht = htp.tile([128, NQT, DS, 128], F16, tag="htp")
        wlo = wlop.tile([128, NDO, DS, 128], F16, tag="wlo")
        whi = whip.tile([128, NDO, DS, 128], F16, tag="whi")
        dect = decp.tile([128, NKT, DS, 128], F16, tag="dec")

        load_blocks(wqt[:, 0:2], wq, 0, 2)
        load_blocks(ht[:, 0:4], hsT, 0, 4)
        load_blocks(ht[:, 4:8], hsT, 4, 4)
        load_blocks(wqt[:, 2:NDO], wq, 2, NDO - 2)

        # long-lived SBUF tensors
        QT = qtp.tile([128, DS, QS], F16, tag="QT")      # [d, q] rhs for scores
        KT = ktp.tile([128, DS, KLp], F16, tag="KT")     # [d, k] lhsT for scores
        V = vp.tile([128, NKT, Dp], F16, tag="V")        # [k, d] rhs for AV

        # ---------------- Phase B: QT[do, q] = Wq^T @ hsT ------------------
        for do in range(NDO):
            # bulk prefetch for A1/A2 behind B's compute
            if do == 1:
                load_blocks(wlo[:], wkv, 0, NDO)
            elif do == 3:
                load_blocks(dect[:, 0:8], decT, 0, 8)
            elif do == 5:
                load_blocks(dect[:, 8:16], decT, 8, 8)
            elif do == 7:
                load_blocks(whi[:], wkv, NDO, NDO)
            ps0 = psP.tile([128, 512], F32, tag="psP")
            ps1 = psP.tile([128, 512], F32, tag="psP")
            for di in range(DS):
                nc.tensor.matmul(
                    ps0[:], wqt[:, do, di, :], ht[:, 0:4, di, :],
                    start=(di == 0), stop=(di == DS - 1),
                )
                nc.tensor.matmul(
                    ps1[:], wqt[:, do, di, :], ht[:, 4:8, di, :],
                    start=(di == 0), stop=(di == DS - 1),
                )
            nc.vector.tensor_copy(QT[:, do, 0:512], ps0[:])
            nc.vector.tensor_copy(QT[:, do, 512:1024], ps1[:])

        # ---------------- Phase A1: KT[do, k] = Wkv_lo^T @ decT ------------
        for g in range(2):           # kc-pair subphases: dec halves
            for do in range(NDO):
                ps0 = psP.tile([128, 512], F32, tag="psP")
                ps1 = psP.tile([128, 512], F32, tag="psP")
                for di in range(DS):
                    nc.tensor.matmul(
                        ps0[:], wlo[:, do, di, :], dect[:, 8 * g : 8 * g + 4, di, :],
                        start=(di == 0), stop=(di == DS - 1),
                    )
                    nc.tensor.matmul(
                        ps1[:], wlo[:, do, di, :], dect[:, 8 * g + 4 : 8 * g + 8, di, :],
                        start=(di == 0), stop=(di == DS - 1),
                    )
                nc.vector.tensor_copy(
                    KT[:, do, 1024 * g : 1024 * g + 512], ps0[:]
                )
                nc.vector.tensor_copy(
                    KT[:, do, 1024 * g + 512 : 1024 * g + 1024], ps1[:]
                )

        # ---------------- Phase A2: V[k, d] = decT^T @ Wkv_hi --------------
        for kt in range(NKT):
            ps0 = psP.tile([128, 512], F32, tag="psP")
            ps1 = psP.tile([128, 512], F32, tag="psP")
            for di in range(DS):
                nc.tensor.matmul(
                    ps0[:], dect[:, kt, di, :], whi[:, 0:4, di, :],
                    start=(di == 0), stop=(di == DS - 1),
                )
                nc.tensor.matmul(
                    ps1[:], dect[:, kt, di, :], whi[:, 4:8, di, :],
                    start=(di == 0), stop=(di == DS - 1),
                )
            nc.vector.tensor_copy(V[:, kt, 0:512], ps0[:])
            nc.vector.tensor_copy(V[:, kt, 512:1024], ps1[:])
        close(psP_cm)
        close(decp_cm)
        close(whip_cm)
        close(wlop_cm)
        close(htp_cm)
        close(wqp_cm)

        # ---------------- Phase C: attention ------------------------------
        ptp_cm = tc.tile_pool(name="ptp", bufs=NQC, side="right")
        statp_cm = tc.tile_pool(name="stat", bufs=4, side="right")
        ostp_cm = tc.tile_pool(name="ost", bufs=2, side="right")
        ps_sc_cm = tc.tile_pool(name="ps_sc", bufs=3, space="PSUM")
        ps_av_cm = tc.tile_pool(name="ps_av", bufs=4, space="PSUM")
        ls_cm = tc.tile_pool(name="ls", bufs=1, space="PSUM")
        ptp = enter(ptp_cm)
        statp = enter(statp_cm)
        ostp = enter(ostp_cm)
        ps_sc = enter(ps_sc_cm)
        ps_av = enter(ps_av_cm)
        lsp = enter(ls_cm)
        PT = [
            ptp.tile([128, NKT, 512], F16, tag="ptp", name=f"PT{c}")
            for c in range(NQC)
        ]
        ls = lsp.tile([128, 64], F32, tag="ls")

        # scores^T + exp for both q-chunks: S^T[k, q] = K @ Q^T
        for kt in range(NKT):
            pscs = []
            for c in range(NQC):
                ps = ps_sc.tile([128, 512], F32, tag="ps_sc")
                pscs.append(ps)
            for di in range(DS):
                for c in range(NQC):
                    nc.tensor.matmul(
                        pscs[c][:],
                        KT[:, di, 128 * kt : 128 * kt + 128],
                        QT[:, di, 512 * c : 512 * c + 512],
                        start=(di == 0), stop=(di == DS - 1),
                    )
            for c in range(NQC):
                nc.scalar.activation(
                    PT[c][:, kt, :], pscs[c][:], EXP,
                    bias=0.0, scale=float(scale),
                )

        # AV + row-sum + normalize per 128-q subtile
        for c in range(NQC):
            for qs in range(4):
                av0 = ps_av.tile([128, 512], F32, tag="ps_av")
                av1 = ps_av.tile([128, 512], F32, tag="ps_av")
                col = 8 * (4 * c + qs)
                for kt in range(NKT):
                    lhsT = PT[c][:, kt, 128 * qs : 128 * qs + 128]
                    nc.tensor.matmul(
                        av0[:], lhsT, V[:, kt, 0:512],
                        start=(kt == 0), stop=(kt == NKT - 1),
                    )
                    nc.tensor.matmul(
                        av1[:], lhsT, V[:, kt, 512:1024],
                        start=(kt == 0), stop=(kt == NKT - 1),
                    )
                    nc.tensor.matmul(
                        ls[:, col : col + 1], lhsT, ones[:, 0:1],
                        start=(kt == 0), stop=(kt == NKT - 1),
                    )
                recip = statp.tile([128, 1], F32, tag="stat")
                nc.vector.reciprocal(recip[:], ls[:, col : col + 1])
                ot = ostp.tile([128, Dp], F32, tag="ost")
                nc.scalar.activation(
                    ot[:, 0:512], av0[:], ACOPY, bias=0.0, scale=recip[:],
                )
                nc.scalar.activation(
                    ot[:, 512:1024], av1[:], ACOPY, bias=0.0, scale=recip[:],
                )
                qrow = (4 * c + qs) * 128
                nc.sync.dma_start(out[qrow : qrow + 128, :], ot[:])

        for cm in list(reversed(pools)):
            close(cm)

    legalize_waits(nc)
    return nc


def _pack_dT_blocks(x, DS):
    """[N, Dp] -> [N//128, 128, DS*128] where block b holds
    res[b, p, s*128+o] = x[b*128+o, s*128+p]  (partitions carry d, free
    carries (subtile s, n-within-block)).  fp16 output."""
    N, Dp = x.shape
    r = x.reshape(N // 128, 128, DS, 128).transpose(0, 3, 2, 1)
    return np.ascontiguousarray(r.reshape(N // 128, 128, DS * 128).astype(np.float16))


def prepare_in_maps(hidden_states, decoder_hidden_states, Wq, Wkv):
    hidden_states = np.asarray(hidden_states, dtype=np.float32)
    decoder_hidden_states = np.asarray(decoder_hidden_states, dtype=np.float32)
    Wq = np.asarray(Wq, dtype=np.float32)
    Wkv = np.asarray(Wkv, dtype=np.float32)
    QS = QL // 2
    DS = D // 128

    wq_p = _pack_dT_blocks(Wq.T, DS)      # [do][p, s*128+o] = Wq[s*128+p, do*128+o]
    wkv_p = _pack_dT_blocks(Wkv.T, DS)

    in_maps = []
    for c in range(N_CORES):
        b, h = c // 2, c % 2
        hs = hidden_states[b, h * QS : (h + 1) * QS]        # [QS, D]
        dec = decoder_hidden_states[b]                      # [KL, D]
        in_maps.append(
            {
                "hsT": _pack_dT_blocks(hs, DS),    # [NQT, 128, DS*128]
                "decT": _pack_dT_blocks(dec, DS),  # [NKT, 128, DS*128]
                "wq": wq_p,
                "wkv": wkv_p,
            }
        )
    return in_maps


def kernel(hidden_states, decoder_hidden_states, Wq, Wkv):
    QS = QL // 2
    scale = 1.0 / float(np.sqrt(D))

    nc = bass.Bass()
    build_attention(nc, QS, KL, D, scale)
    in_maps = prepare_in_maps(hidden_states, decoder_hidden_states, Wq, Wkv)

    res = run_bass_kernel_spmd(nc, in_maps, list(range(N_CORES)))

    out = np.empty((B, QL, D), dtype=np.float32)
    for c in range(N_CORES):
        b, h = c // 2, c % 2
        out[b, h * QS : (h + 1) * QS] = res.results[c]["out"]
    return out


# revision 10
# speedup vs baseline: 1.5103x; 1.0488x over previous
"""Trainium2 Bass kernel for CrossAttention (B=4, QL=KL=2048, D=1024, fp32).

reference:
    query = hidden_states @ Wq                      # [B, QL, D]
    kv    = decoder_hidden_states @ Wkv             # [B, KL, 2D]
    key, value = split(kv, 2, axis=-1)
    scores = einsum('bqd,bkd->bqk', query, key) / sqrt(D)
    w = softmax(scores, axis=-1)
    out = einsum('bqk,bkd->bqd', w, value)          # [B, QL, D]

Sharding: 8 cores = batch(4) x q-half(2).  Each core owns 1024 query rows of
one batch.  The K/V projection for a batch is split between the two cores of
the pair by key rows (each computes 1024 of 2048 keys) and the halves are
exchanged with a pair-wise HBM AllGather, so no projection work is
duplicated.  The program is parity-agnostic: the host feeds each core only
its half of decT, the core computes its local K/V half, and both halves are
read back from the AllGather output (replica-group rank order == global key
order).

All data is fp16 (11-bit mantissa: ~0.05% quantization error, well inside the
2e-2 gate).  fp16 matmuls stream at 1 row/cycle like fp32r but their weight
loads use the fast-weight-load path and half the SBUF/DMA footprint, so
everything stays resident in SBUF.

Attention is computed with TRANSPOSED scores: S^T[k, q] = K @ Q^T directly
via lhsT=KT-slice [d,k], rhs=QT [d, q-512].  exp(S^T) on ACT lands in exactly
the [k, q] stationary layout the AV matmul needs, so there are NO DVE
transposes and NO casts anywhere.  Row sums for softmax come from one extra
1-column matmul against a ones vector that reuses the AV stationary, and
normalization is folded into the ACT copy that drains AV psum.

Phase order: warmup -> A1 (local K proj, then K AllGather) -> A2 (local V
proj, then V AllGather) -> B (Q proj) -> C (scores both q-chunks, then AV).
DMA is critical-first: A1's inputs, then bulk prefetch behind compute; the
exchange bounce DMAs are queued after all input loads so they never stall
the ring.

This walrus build allows only ONE embedded semaphore wait per hardware
instruction; legalize_waits() splits any extra waits onto injected
same-engine NOPs after Tile scheduling.
"""

import sys

if "/opt/trn_rl_repo" not in sys.path:
    sys.path.insert(0, "/opt/trn_rl_repo")

import numpy as np

import bass_rust
import concourse.bass as bass
import concourse.mybir as mybir
import concourse.tile as tile
from concourse.bass_utils import run_bass_kernel_spmd

F32 = mybir.dt.float32
F16 = mybir.dt.float16
EXP = mybir.ActivationFunctionType.Exp
ACOPY = mybir.ActivationFunctionType.Copy

N_CORES = 8
B, QL, KL, D = 4, 2048, 2048, 1024
PAIRS = [[2 * i, 2 * i + 1] for i in range(N_CORES // 2)]


def legalize_waits(nc, max_waits=1):
    """TRN2 instructions embed at most one semaphore wait.  Move excess waits
    emitted by Tile onto same-engine NOPs inserted just before the owning
    instruction (engine FIFO makes this semantically identical)."""
    cnt = 0
    for fn in nc.m.functions:
        for bb in fn.blocks:
            out = []
            changed = False
            for ins in bb.instructions:
                si = ins.sync_info
                if si is not None and si.on_wait and len(si.on_wait) > max_waits:
                    waits = list(si.on_wait)
                    for w in waits[:-max_waits]:
                        cnt += 1
                        nop = bass_rust.InstNoOp(name=f"I-wfix-{cnt}")
                        nop.engine = ins.engine
                        nop.sync_info = mybir.SyncInfo(on_wait=[w], on_update=[])
                        out.append(nop)
                    ins.sync_info = mybir.SyncInfo(
                        on_wait=waits[-max_waits:],
                        on_update=list(si.on_update or []),
                    )
                    changed = True
                out.append(ins)
            if changed:
                bb.instructions = out
    return cnt


def build_attention(nc, QS, KLp, Dp, scale):
    DS = Dp // 128          # contraction subtiles
    NDO = Dp // 128         # output-d 128-chunks
    NKT = KLp // 128        # k 128-chunks
    NKL = NKT // 2          # local k 128-chunks (half of the batch's keys)
    NQT = QS // 128         # q 128-chunks
    NQC = QS // 512         # q 512-chunks
    KLH = KLp // 2          # local key count
    BLK = DS * 128          # free extent of one [128, DS*128] DRAM block

    # block-layout params: [nblk, 128, DS*128], fp16.  decT holds only this
    # core's half of the batch's keys (host slices per core parity).
    hsT = nc.declare_dram_parameter("hsT", [NQT, 128, BLK], F16, isOutput=False)
    decT = nc.declare_dram_parameter("decT", [NKL, 128, BLK], F16, isOutput=False)
    wq = nc.declare_dram_parameter("wq", [NDO, 128, BLK], F16, isOutput=False)
    wkv = nc.declare_dram_parameter("wkv", [2 * NDO, 128, BLK], F16, isOutput=False)
    out = nc.declare_dram_parameter("out", [QS, Dp], F32, isOutput=True)

    def load_blocks(dst, src, blk0, nblk):
        """One DMA moving nblk consecutive [128, BLK] DRAM blocks into an
        SBUF tile laid out [128, nblk, DS, 128]."""
        if nblk == 1:
            nc.sync.dma_start(
                dst[:], src[blk0].rearrange("p (s o) -> p s o", o=128)
            )
        else:
            nc.sync.dma_start(
                dst.rearrange("p b s o -> p b (s o)"),
                src[blk0 : blk0 + nblk].rearrange("b p f -> p b f"),
            )

    with tile.TileContext(nc) as tc:
        pools = []

        def enter(cm):
            pools.append(cm)
            return cm.__enter__()

        def close(cm):
            pools.remove(cm)
            cm.__exit__(None, None, None)

        # long-lived pools on the RIGHT stack; transient per-phase pools on
        # the LEFT stack close LIFO at phase boundaries.
        constp_cm = tc.tile_pool(name="const", bufs=1, side="right")
        ktp_cm = tc.tile_pool(name="ktp", bufs=1, side="right")
        vp_cm = tc.tile_pool(name="vp", bufs=1, side="right")
        qtp_cm = tc.tile_pool(name="qtp", bufs=1, side="right")
        dramp_cm = tc.tile_pool(name="dram", bufs=1, space="DRAM")

        wqp_cm = tc.tile_pool(name="wqp", bufs=1)
        htp_cm = tc.tile_pool(name="htp", bufs=1)
        wlop_cm = tc.tile_pool(name="wlo", bufs=1)
        whip_cm = tc.tile_pool(name="whi", bufs=1)
        decp_cm = tc.tile_pool(name="dec", bufs=1)

        psP_cm = tc.tile_pool(name="psP", bufs=4, space="PSUM")

        constp = enter(constp_cm)
        ktp = enter(ktp_cm)
        vp = enter(vp_cm)
        qtp = enter(qtp_cm)
        dramp = enter(dramp_cm)
        wqp = enter(wqp_cm)
        htp = enter(htp_cm)
        wlop = enter(wlop_cm)
        whip = enter(whip_cm)
        decp = enter(decp_cm)
        psP = enter(psP_cm)

        # constants: ones column (for row sums) + warmup tile
        ones = constp.tile([128, 8], F16)
        warm = constp.tile([128, 640], F16)
        nc.gpsimd.memset(ones[:], 1.0)
        nc.gpsimd.memset(warm[:, 0:128], 1.0)
        nc.vector.memset(warm[:, 128:640], 1.0)

        # HAM warmup: keep the PE busy during the initial DMA wave so the
        # clock gate is at 8/8 when phase A1's first real matmul issues.
        warm_ps_cm = tc.tile_pool(name="wps", bufs=1, space="PSUM")
        warm_ps_pool = enter(warm_ps_cm)
        warm_ps = warm_ps_pool.tile([128, 512], F32)
        for _ in range(45):
            nc.tensor.matmul(
                warm_ps[:], warm[:, 0:128], warm[:, 128:640],
                start=True, stop=True, skip_group_check=True,
            )
        close(warm_ps_cm)

        # input SBUF tiles
        wqt = wqp.tile([128, NDO, DS, 128], F16, tag="wqp")
        ht = htp.tile([128, NQT, DS, 128], F16, tag="htp")
        wlo = wlop.tile([128, NDO, DS, 128], F16, tag="wlo")
        whi = whip.tile([128, NDO, DS, 128], F16, tag="whi")
        dect = decp.tile([128, NKL, DS, 128], F16, tag="dec")

        # critical-first loads: A1's inputs, then everything else.  All
        # input loads are queued before the exchange bounce DMAs so the
        # (FIFO) ring never stalls on a compute dependency.
        load_blocks(dect[:, 0:4], decT, 0, 4)
        load_blocks(wlo[:], wkv, 0, NDO)
        load_blocks(dect[:, 4:8], decT, 4, 4)
        load_blocks(whi[:], wkv, NDO, NDO)
        load_blocks(wqt[:], wq, 0, NDO)
        load_blocks(ht[:, 0:4], hsT, 0, 4)
        load_blocks(ht[:, 4:8], hsT, 4, 4)

        # long-lived SBUF tensors
        QT = qtp.tile([128, DS, QS], F16, tag="QT")      # [d, q] rhs for scores
        KT = ktp.tile([128, DS, KLp], F16, tag="KT")     # [d, k] lhsT for scores
        V = vp.tile([128, NKT, Dp], F16, tag="V")        # [k, d] rhs for AV

        # DRAM bounce buffers for the pair AllGather (HBM-HBM collective)
        kt_in_b = dramp.tile([128, DS, KLH], F16, tag="ktib")
        kt_out_b = dramp.tile([2, 128, DS, KLH], F16, tag="ktob")
        v_in_b = dramp.tile([128, NKL, Dp], F16, tag="vib")
        v_out_b = dramp.tile([2, 128, NKL, Dp], F16, tag="vob")

        # -------- Phase A1: local KT[do, k] = Wkv_lo^T @ decT_local --------
        # local half lands in KT[:, :, 0:KLH]; full KT is written back from
        # the AllGather output below.
        for kc in range(2):
            for do in range(NDO):
                ps = psP.tile([128, 512], F32, tag="psP")
                for di in range(DS):
                    nc.tensor.matmul(
                        ps[:], wlo[:, do, di, :],
                        dect[:, 4 * kc : 4 * kc + 4, di, :],
                        start=(di == 0), stop=(di == DS - 1),
                    )
                nc.vector.tensor_copy(
                    KT[:, do, 512 * kc : 512 * kc + 512], ps[:]
                )
        # K exchange: local half -> bounce -> AllGather -> full KT
        nc.sync.dma_start(kt_in_b[:], KT[:, :, 0:KLH])
        nc.gpsimd.collective_compute(
            "AllGather", mybir.AluOpType.bypass,
            replica_groups=PAIRS,
            ins=[kt_in_b[:]], outs=[kt_out_b[:]],
        )
        nc.sync.dma_start(
            KT.rearrange("p s (j k) -> p s j k", j=2),
            kt_out_b.rearrange("j p s k -> p s j k"),
        )

        # -------- Phase A2: local V[k, d] = decT_local^T @ Wkv_hi ----------
        for kt in range(NKL):
            ps0 = psP.tile([128, 512], F32, tag="psP")
            ps1 = psP.tile([128, 512], F32, tag="psP")
            for di in range(DS):
                nc.tensor.matmul(
                    ps0[:], dect[:, kt, di, :], whi[:, 0:4, di, :],
                    start=(di == 0), stop=(di == DS - 1),
                )
                nc.tensor.matmul(
                    ps1[:], dect[:, kt, di, :], whi[:, 4:8, di, :],
                    start=(di == 0), stop=(di == DS - 1),
                )
            nc.vector.tensor_copy(V[:, kt, 0:512], ps0[:])
            nc.vector.tensor_copy(V[:, kt, 512:1024], ps1[:])
        # V exchange
        nc.sync.dma_start(v_in_b[:], V[:, 0:NKL, :])
        nc.gpsimd.collective_compute(
            "AllGather", mybir.AluOpType.bypass,
            replica_groups=PAIRS,
            ins=[v_in_b[:]], outs=[v_out_b[:]],
        )
        nc.sync.dma_start(
            V.rearrange("p (j t) d -> p j t d", j=2),
            v_out_b.rearrange("j p t d -> p j t d"),
        )

        # ---------------- Phase B: QT[do, q] = Wq^T @ hsT ------------------
        for do in range(NDO):
            ps0 = psP.tile([128, 512], F32, tag="psP")
            ps1 = psP.tile([128, 512], F32, tag="psP")
            for di in range(DS):
                nc.tensor.matmul(
                    ps0[:], wqt[:, do, di, :], ht[:, 0:4, di, :],
                    start=(di == 0), stop=(di == DS - 1),
                )
                nc.tensor.matmul(
                    ps1[:], wqt[:, do, di, :], ht[:, 4:8, di, :],
                    start=(di == 0), stop=(di == DS - 1),
                )
            nc.vector.tensor_copy(QT[:, do, 0:512], ps0[:])
            nc.vector.tensor_copy(QT[:, do, 512:1024], ps1[:])
        close(psP_cm)
        close(decp_cm)
        close(whip_cm)
        close(wlop_cm)
        close(htp_cm)
        close(wqp_cm)

        # ---------------- Phase C: attention ------------------------------
        ptp_cm = tc.tile_pool(name="ptp", bufs=NQC, side="right")
        statp_cm = tc.tile_pool(name="stat", bufs=4, side="right")
        ostp_cm = tc.tile_pool(name="ost", bufs=2, side="right")
        ps_sc_cm = tc.tile_pool(name="ps_sc", bufs=3, space="PSUM")
        ps_av_cm = tc.tile_pool(name="ps_av", bufs=4, space="PSUM")
        ls_cm = tc.tile_pool(name="ls", bufs=1, space="PSUM")
        ptp = enter(ptp_cm)
        statp = enter(statp_cm)
        ostp = enter(ostp_cm)
        ps_sc = enter(ps_sc_cm)
        ps_av = enter(ps_av_cm)
        lsp = enter(ls_cm)
        PT = [
            ptp.tile([128, NKT, 512], F16, tag="ptp", name=f"PT{c}")
            for c in range(NQC)
        ]
        ls = lsp.tile([128, 64], F32, tag="ls")

        # scores^T + exp for both q-chunks: S^T[k, q] = K @ Q^T
        for kt in range(NKT):
            pscs = []
            for c in range(NQC):
                ps = ps_sc.tile([128, 512], F32, tag="ps_sc")
                pscs.append(ps)
            for di in range(DS):
                for c in range(NQC):
                    nc.tensor.matmul(
                        pscs[c][:],
                        KT[:, di, 128 * kt : 128 * kt + 128],
                        QT[:, di, 512 * c : 512 * c + 512],
                        start=(di == 0), stop=(di == DS - 1),
                    )
            for c in range(NQC):
                nc.scalar.activation(
                    PT[c][:, kt, :], pscs[c][:], EXP,
                    bias=0.0, scale=float(scale),
                )

        # AV + row-sum + normalize per 128-q subtile
        for c in range(NQC):
            for qs in range(4):
                last = c == NQC - 1 and qs == 3
                av0 = ps_av.tile([128, 512], F32, tag="ps_av")
                av1 = ps_av.tile([128, 512], F32, tag="ps_av")
                col = 8 * (4 * c + qs)
                recip = statp.tile([128, 1], F32, tag="stat")
                ot = ostp.tile([128, Dp], F32, tag="ost")
                qrow = (4 * c + qs) * 128

                def pt_lhsT(kt, c=c, qs=qs):
                    return PT[c][:, kt, 128 * qs : 128 * qs + 128]

                if not last:
                    for kt in range(NKT):
                        lhsT = pt_lhsT(kt)
                        nc.tensor.matmul(
                            av0[:], lhsT, V[:, kt, 0:512],
                            start=(kt == 0), stop=(kt == NKT - 1),
                        )
                        nc.tensor.matmul(
                            av1[:], lhsT, V[:, kt, 512:1024],
                            start=(kt == 0), stop=(kt == NKT - 1),
                        )
                        nc.tensor.matmul(
                            ls[:, col : col + 1], lhsT, ones[:, 0:1],
                            start=(kt == 0), stop=(kt == NKT - 1),
                        )
                    nc.vector.reciprocal(recip[:], ls[:, col : col + 1])
                    nc.scalar.activation(
                        ot[:, 0:512], av0[:], ACOPY, bias=0.0, scale=recip[:],
                    )
                    nc.scalar.activation(
                        ot[:, 512:1024], av1[:], ACOPY, bias=0.0, scale=recip[:],
                    )
                    nc.sync.dma_start(out[qrow : qrow + 128, :], ot[:])
                else:
                    # last subtile: row-sum matmuls first so the reciprocal
                    # computes during AV, and av0 drains + ships while av1 is
                    # still accumulating — shortens the kernel tail.
                    for kt in range(NKT):
                        nc.tensor.matmul(
                            ls[:, col : col + 1], pt_lhsT(kt), ones[:, 0:1],
                            start=(kt == 0), stop=(kt == NKT - 1),
                        )
                    nc.vector.reciprocal(recip[:], ls[:, col : col + 1])
                    for kt in range(NKT):
                        nc.tensor.matmul(
                            av0[:], pt_lhsT(kt), V[:, kt, 0:512],
                            start=(kt == 0), stop=(kt == NKT - 1),
                        )
                    nc.scalar.activation(
                        ot[:, 0:512], av0[:], ACOPY, bias=0.0, scale=recip[:],
                    )
                    nc.sync.dma_start(out[qrow : qrow + 128, 0:512], ot[:, 0:512])
                    for kt in range(NKT):
                        nc.tensor.matmul(
                            av1[:], pt_lhsT(kt), V[:, kt, 512:1024],
                            start=(kt == 0), stop=(kt == NKT - 1),
                        )
                    nc.scalar.activation(
                        ot[:, 512:1024], av1[:], ACOPY, bias=0.0, scale=recip[:],
                    )
                    nc.sync.dma_start(
                        out[qrow : qrow + 128, 512:1024], ot[:, 512:1024]
                    )

        for cm in list(reversed(pools)):
            close(cm)

    legalize_waits(nc)
    return nc


def _pack_dT_blocks(x, DS):
    """[N, Dp] -> [N//128, 128, DS*128] where block b holds
    res[b, p, s*128+o] = x[b*128+o, s*128+p]  (partitions carry d, free
    carries (subtile s, n-within-block)).  fp16 output."""
    N, Dp = x.shape
    r = x.reshape(N // 128, 128, DS, 128).transpose(0, 3, 2, 1)
    return np.ascontiguousarray(r.reshape(N // 128, 128, DS * 128).astype(np.float16))


def prepare_in_maps(hidden_states, decoder_hidden_states, Wq, Wkv):
    hidden_states = np.asarray(hidden_states, dtype=np.float32)
    decoder_hidden_states = np.asarray(decoder_hidden_states, dtype=np.float32)
    Wq = np.asarray(Wq, dtype=np.float32)
    Wkv = np.asarray(Wkv, dtype=np.float32)
    QS = QL // 2
    KLH = KL // 2
    DS = D // 128

    wq_p = _pack_dT_blocks(Wq.T, DS)      # [do][p, s*128+o] = Wq[s*128+p, do*128+o]
    wkv_p = _pack_dT_blocks(Wkv.T, DS)

    in_maps = []
    for c in range(N_CORES):
        b, h = c // 2, c % 2
        hs = hidden_states[b, h * QS : (h + 1) * QS]          # [QS, D]
        dec = decoder_hidden_states[b, h * KLH : (h + 1) * KLH]  # local key half
        in_maps.append(
            {
                "hsT": _pack_dT_blocks(hs, DS),    # [NQT, 128, DS*128]
                "decT": _pack_dT_blocks(dec, DS),  # [NKL, 128, DS*128]
                "wq": wq_p,
                "wkv": wkv_p,
            }
        )
    return in_maps


def kernel(hidden_states, decoder_hidden_states, Wq, Wkv):
    QS = QL // 2
    scale = 1.0 / float(np.sqrt(D))

    nc = bass.Bass(num_devices=N_CORES)
    build_attention(nc, QS, KL, D, scale)
    in_maps = prepare_in_maps(hidden_states, decoder_hidden_states, Wq, Wkv)

    res = run_bass_kernel_spmd(nc, in_maps, list(range(N_CORES)))

    out = np.empty((B, QL, D), dtype=np.float32)
    for c in range(N_CORES):
        b, h = c // 2, c % 2
        out[b, h * QS : (h + 1) * QS] = res.results[c]["out"]
    return out
